# revision 16
# baseline (speedup 1.0000x reference)
"""GAT layer kernel for Trainium2 (8 NeuronCores, SPMD, no collectives).

Math (reference):
    att = h @ h.T / sqrt(256)
    A = softmax(where(adj>0, att, -9e15), axis=1)
    A = (A + I) * 0.5; rows < k (k = nnz(adj[:,0])) overwritten with I
    out = relu(A @ (h @ W.T + b))

v2 algorithm (fp8 DoubleRow matmuls + exp-as-fp8-bit-encode):
  - rows [0,k): out = relu(h@W.T + b)  (identity rows)
  - rows [k,N): out = relu(num*(0.5/S_c) + g2*(1 + d/S_c)), where
        num[i,:] = sum_j em[j,i] * hnew8[j,:],  S = sum_j em[j,i],
        S_c = S + d_i,  g2 = 0.5*(h@W.T) + 0.5*b,
        d_i = host-computed exact diag term (m_ii * e^att_ii / 2).
  - em[j,i] ~= e^att[j,i]/2 stored as fp8e5 BIT PATTERNS: the PE computes
    satt = 5.7708*att + 56 directly (h8 pre-scaled by sqrt(5.7708/16);
    feature dim 255 is sacrificed for a constant bias row), and the u8
    bits b = satt decode in e5m2 as 2^((b-60)/4) = e^att * 2^-1.  The
    diagonal is excluded via the mask and re-blended exactly from host d.
  - att matmul: fp8e4 DoubleRow (K=256 in one pass, 0.5 cyc/col);
    second matmul: fp8e5 DoubleRow over j-chunk pairs.
  - mask fused into the encode, split across engines per j-chunk:
      class A (DVE):      em_u8 = min(att_psum, maskC{0,123})
      class B (ACT+Pool): em_u8 = Copy(att_psum)->u8; em *= m01{0,1}
  - hnew8 = fp8e5(h@W.T + b) precomputed host-side, streamed from DRAM.

Sharding: identity rows and attention rows each split evenly across the 8
cores; every core runs the same NEFF on different input slices.
"""

import math
import os
import sys

for _p in ("/opt/trn_rl_repo", "/root/.axon_site/_ro/trn_rl_repo"):
    if os.path.isdir(_p) and _p not in sys.path:
        sys.path.append(_p)

import numpy as np
import orjson

import concourse.bass as bass
import concourse.tile as tile
from concourse import mybir

F32 = mybir.dt.float32
F16 = mybir.dt.float16
BF16 = mybir.dt.bfloat16
F8E4 = mybir.dt.float8e4
F8E5 = mybir.dt.float8e5
I8 = mybir.dt.int8
U8 = mybir.dt.uint8
DR = mybir.MatmulPerfMode.DoubleRow

N = 8192
D = 256
NCORES = 8
NJC = N // 128  # 64 j-chunks
SLOPE = 8.0 / np.log(2.0) / 2.0  # 5.7708: e5m2 bit-steps per unit att
ALPHA = float(np.sqrt(SLOPE / 16.0))  # h8 pre-scale
CBIAS = 56.0  # bias row constant: bits = 5.7708*att + 56
# per-jc engine class: True -> class A (DVE fused min), False -> B (ACT+Pool)
CLS_A = [(jc % 2 == 0) or ((jc // 2) % 4 == 3) for jc in range(NJC)]


def _spill_waits(nc, max_sync=2):
    """Walrus rejects instructions with more sync commands than the lowered
    ISA struct can hold (2 for compute/DMA, 1 for NoOp/Drain/Ldweights).
    Move excess waits onto injected NoOps preceding the instruction."""
    bir = orjson.loads(nc.to_json_bytes())
    for fn in bir["functions"]:
        for bb in fn["blocks"]:
            insts = bb.get("instructions") or []
            out = []
            for inst in insts:
                si = inst.get("sync_info")
                if si:
                    waits = si.get("on_wait") or []
                    upds = si.get("on_update") or []
                    op = inst["opcode"]
                    lim = 1 if op in ("NoOp", "Drain", "Ldweights") else max_sync
                    cap = max(0, lim - len(upds))
                    if len(waits) > cap:
                        extra = waits[cap:]
                        si["on_wait"] = waits[:cap]
                        for ci, w in enumerate(extra):
                            out.append(
                                {
                                    "engine": inst["engine"],
                                    "ins": [],
                                    "outs": [],
                                    "name": f"{inst['name']}-sw{ci}",
                                    "opcode": "NoOp",
                                    "sync_info": {"on_wait": [w], "on_update": []},
                                    "debug": inst.get("debug", 0),
                                }
                            )
                out.append(inst)
            bb["instructions"] = out
    blob = orjson.dumps(bir)
    nc.to_json_bytes = lambda: blob


def _build(nid, nis, depth=3):
    """SPMD program. nid/nis = number of 128-row identity / attention
    sub-tiles per core. own = (nid+nis)*128 rows per core."""
    nown = nid + nis
    own = nown * 128
    rpad = nis * 128

    nc = bass.Bass("TRN2", target_bir_lowering=False, debug=False, num_devices=1)

    hx_d = nc.dram_tensor("hx", [128, 2 * N], U8, kind="ExternalInput").ap()
    hxo_d = nc.dram_tensor("hxo", [128, 2 * rpad], U8, kind="ExternalInput").ap()
    hTo_d = nc.dram_tensor("hTo", [D, own], F16, kind="ExternalInput").ap()
    WT_d = nc.dram_tensor("WT", [D, 256], F16, kind="ExternalInput").ap()
    bv_d = nc.dram_tensor("bv", [1, 256], F16, kind="ExternalInput").ap()
    hn_d = nc.dram_tensor("hn", [128, (NJC // 2) * 514], U8, kind="ExternalInput").ap()
    mT_d = nc.dram_tensor("mT", [128, NJC * rpad], I8, kind="ExternalInput").ap()
    dv_d = nc.dram_tensor("dv", [128, nis], F32, kind="ExternalInput").ap()
    out_d = nc.dram_tensor("out", [own, 256], BF16, kind="ExternalOutput").ap()

    with tile.TileContext(nc) as tc:
        with (
            tc.tile_pool(name="big", bufs=1) as big,
            tc.tile_pool(name="work", bufs=2) as work,
            tc.tile_pool(name="fin", bufs=2) as fin,
            tc.tile_pool(name="app", bufs=1, space="PSUM") as app,
            tc.tile_pool(name="accp", bufs=1, space="PSUM") as accp,
        ):
            # --- persistent loads (ordered for pipelining; separate tiles
            # per chunk because Tile dependency tracking is tile-granular) ---
            # hx j-chunks: [512, 3584, 4096]; mask jc-chunks; hn pair-chunks
            HXC = [(0, 512), (512, 4096), (4096, 8192)]
            MC = [(0, 2), (2, 8), (8, 16), (16, 24), (24, 32), (32, 40), (40, 48), (48, 56), (56, 64)]
            HC = [(0, 4), (4, 12), (12, 22), (22, 32)]
            hx_ts = {}
            mT_ts = {}
            hn_ts = {}

            def load_hx(ci):
                lo, hi = HXC[ci]
                t = big.tile([128, 2 * (hi - lo)], U8, tag=f"hx{ci}")
                nc.sync.dma_start(t[:], hx_d[:, 2 * lo : 2 * hi])
                hx_ts[ci] = t

            def load_m(ci):
                lo, hi = MC[ci]
                t = big.tile([128, (hi - lo) * rpad], I8, tag=f"mT{ci}")
                nc.sync.dma_start(t[:], mT_d[:, lo * rpad : hi * rpad])
                mT_ts[ci] = t

            def load_hn(ci):
                lo, hi = HC[ci]
                t = big.tile([128, (hi - lo) * 514], U8, tag=f"hn{ci}")
                nc.sync.dma_start(t[:], hn_d[:, lo * 514 : hi * 514])
                hn_ts[ci] = t

            load_hx(0)
            hxo_t = big.tile([128, 2 * rpad], U8, tag="hxo")
            nc.sync.dma_start(hxo_t[:], hxo_d[:, :])
            load_m(0)
            load_hx(1)
            load_m(1)
            load_hn(0)
            load_m(2)
            load_hx(2)
            load_hn(1)
            load_m(3)
            hTo_t = []
            WT_t = []
            for dchunk in range(2):
                t = big.tile([128, own], F16, tag=f"hTo{dchunk}")
                nc.sync.dma_start(t[:], hTo_d[dchunk * 128 : (dchunk + 1) * 128, :])
                hTo_t.append(t)
                t = big.tile([128, 256], F16, tag=f"WT{dchunk}")
                nc.sync.dma_start(t[:], WT_d[dchunk * 128 : (dchunk + 1) * 128, :])
                WT_t.append(t)
            bv_t = big.tile([1, 256], F16, tag="bv")
            nc.sync.dma_start(bv_t[:], bv_d[:, :])
            dv_t = big.tile([128, nis], F32, tag="dv")
            nc.sync.dma_start(dv_t[:], dv_d[:, :])
            one_row = big.tile([1, 128], F16, tag="onerow")
            nc.vector.memset(one_row[:], 1.0)
            load_m(4)
            load_hn(2)
            load_m(5)
            load_m(6)
            load_hn(3)
            load_m(7)
            load_m(8)

            def hx_slice(jc):
                for ci, (lo, hi) in enumerate(HXC):
                    if lo <= jc * 128 < hi:
                        t = hx_ts[ci]
                        w = hi - lo
                        a3 = t[:].bitcast(F8E4).rearrange("p (t j) -> p t j", t=2)
                        return a3[:, :, jc * 128 - lo : (jc + 1) * 128 - lo]
                raise AssertionError

            def m_slice(jc):
                for ci, (lo, hi) in enumerate(MC):
                    if lo <= jc < hi:
                        return mT_ts[ci][:, (jc - lo) * rpad : (jc - lo + 1) * rpad]
                raise AssertionError

            def m_slice2(pair):
                jc = 2 * pair
                for ci, (lo, hi) in enumerate(MC):
                    if lo <= jc < hi:
                        return mT_ts[ci][:, (jc - lo) * rpad : (jc - lo + 2) * rpad]
                raise AssertionError

            def hn_slice(pair):
                for ci, (lo, hi) in enumerate(HC):
                    if lo <= pair < hi:
                        return hn_ts[ci][:, (pair - lo) * 514 : (pair - lo + 1) * 514]
                raise AssertionError

            ones_t = big.tile([128, 1], F32, tag="ones1")
            nc.vector.memset(ones_t[:], 1.0)
            zer_t = big.tile([128, 256], F32, tag="zer256")
            nc.vector.memset(zer_t[:], 0.0)
            ebias_t = big.tile([128, 1], F32, tag="ebias")
            nc.vector.memset(ebias_t[:], -10.396842)

            hxo3 = hxo_t[:].bitcast(F8E4).rearrange("p (t i) -> p t i", t=2)

            # --- own phase emitter (interleaved into the main loop) ---
            g_t = [None] * nis

            def emit_own(t_i):
                psw = app.tile([128, rpad], F32, tag="att_ps0")
                ps = psw[:, 0:256]
                for dchunk in range(2):
                    nc.tensor.matmul(
                        ps,
                        hTo_t[dchunk][:, t_i * 128 : (t_i + 1) * 128],
                        WT_t[dchunk][:],
                        start=(dchunk == 0),
                        stop=False,
                    )
                nc.tensor.matmul(ps, one_row[:], bv_t[:], start=False, stop=True)
                if t_i < nid:
                    o_t = fin.tile([128, 256], BF16, tag="ido")
                    nc.scalar.activation(
                        o_t[:], ps, mybir.ActivationFunctionType.Relu
                    )
                    nc.sync.dma_start(out_d[t_i * 128 : (t_i + 1) * 128, :], o_t[:])
                else:
                    g = big.tile([128, 256], F32, tag=f"g{t_i - nid}")
                    nc.scalar.activation(
                        g[:], ps, mybir.ActivationFunctionType.Copy, scale=0.5
                    )
                    g_t[t_i - nid] = g

            # --- attention main loop ---
            acc = []
            for s in range(nis):
                acc_t = accp.tile([128, 257], F32, tag=f"acc{s}")
                acc.append(acc_t)
            pend = []

            def emit_second(pair, em_pair):
                em3 = em_pair[:].bitcast(F8E5).rearrange("p (t i) -> p t i", t=2)
                hn3 = hn_slice(pair).bitcast(F8E5).rearrange("p (t f) -> p t f", t=2)
                for s in range(nis):
                    nc.tensor.matmul(
                        acc[s][:],
                        em3[:, :, s * 128 : (s + 1) * 128],
                        hn3,
                        start=(pair == 0),
                        stop=(pair == NJC // 2 - 1),
                        perf_mode=DR,
                    )

            em_t = None
            own_next = 0
            for jc in range(NJC):
                if jc % 6 == 2 and jc >= 8 and own_next < nown:
                    emit_own(own_next)
                    own_next += 1
                half = jc % 2
                if half == 0:
                    em_t = work.tile([128, 2 * rpad], U8, tag=f"em{(jc // 2) % 6}")
                aps = app.tile([128, rpad], F32, tag=f"att_ps{jc % 4}")
                nc.tensor.matmul(
                    aps[:],
                    hx_slice(jc),
                    hxo3,
                    start=True,
                    stop=True,
                    perf_mode=DR,
                )
                em_half = em_t[:, half * rpad : (half + 1) * rpad]
                m_sl = m_slice(jc)
                if CLS_A[jc]:
                    nc.vector.tensor_tensor(
                        em_half, aps[:], m_sl, op=mybir.AluOpType.min
                    )
                else:
                    eb = work.tile([128, rpad], BF16, tag=f"eb{jc % 4}")
                    nc.scalar.activation(
                        eb[:], aps[:], mybir.ActivationFunctionType.Exp,
                        scale=0.17328679, bias=ebias_t[:],
                    )
                    nc.gpsimd.tensor_tensor(
                        em_half.bitcast(F8E5), eb[:], m_sl, op=mybir.AluOpType.mult
                    )
                if half == 1:
                    pend.append((jc // 2, em_t))
                    if len(pend) > depth:
                        emit_second(*pend.pop(0))
            while own_next < nown:
                emit_own(own_next)
                own_next += 1
            for item in pend:
                emit_second(*item)

            # --- finalize per s-tile ---
            for s in range(nis):
                a = acc[s]
                sc = fin.tile([128, 1], F32, tag="sc")
                nc.vector.tensor_tensor(
                    sc[:], a[:, 256:257], dv_t[:, s : s + 1], op=mybir.AluOpType.add
                )
                r = fin.tile([128, 1], F32, tag="r")
                nc.vector.reciprocal(r[:], sc[:])
                r0 = fin.tile([128, 1], F32, tag="r0")
                nc.vector.tensor_scalar_mul(r0[:], r[:], 0.5)
                rd2 = fin.tile([128, 1], F32, tag="rd2")
                nc.vector.scalar_tensor_tensor(
                    rd2[:], dv_t[:, s : s + 1], r[:], ones_t[:],
                    op0=mybir.AluOpType.mult, op1=mybir.AluOpType.add,
                )
                t1 = fin.tile([128, 256], F32, tag="t1")
                nc.scalar.activation(
                    t1[:], g_t[s][:], mybir.ActivationFunctionType.Copy,
                    scale=rd2[:],
                )
                t2 = fin.tile([128, 256], F32, tag="t2")
                nc.vector.scalar_tensor_tensor(
                    t2[:], a[:, 0:256], r0[:], t1[:],
                    op0=mybir.AluOpType.mult, op1=mybir.AluOpType.add,
                )
                o_t = fin.tile([128, 256], BF16, tag="ao")
                nc.vector.tensor_scalar_max(o_t[:], t2[:], 0.0)
                nc.sync.dma_start(
                    out_d[(nid + s) * 128 : (nid + s + 1) * 128, :], o_t[:]
                )

    _spill_waits(nc)
    return nc


_CACHE = {}


def _prepare(h, adj, W, b):
    """Host-side sharding + fp8 encode prep. Returns (nc, in_maps, assemble)."""
    import ml_dtypes

    E4 = ml_dtypes.float8_e4m3fn
    E5 = ml_dtypes.float8_e5m2

    h = np.asarray(h, dtype=np.float32)
    adj = np.asarray(adj)
    W = np.asarray(W, dtype=np.float32)
    b = np.asarray(b, dtype=np.float32)

    k = int(np.count_nonzero(adj[:, 0]))
    nid = (k + NCORES * 128 - 1) // (NCORES * 128)
    nis = (N - k + NCORES * 128 - 1) // (NCORES * 128)
    key = (nid, nis)
    if key not in _CACHE:
        _CACHE[key] = _build(nid, nis)
    nc = _CACHE[key]

    kid = nid * 128
    rpad = nis * 128

    # fp8 h encode, bias row at d=255
    h8q = (ALPHA * h).astype(E4)  # [N, 256]
    h8dec = h8q.astype(np.float32)
    hx = np.empty((N, 256), np.uint8)
    hx[:, :] = h8q.view(np.uint8)
    hx[:, 255] = np.float32(1.0).astype(E4).view(np.uint8).item()
    # device layout: concat of per-chunk [128 p, 2 t, w j] blocks (chunks
    # at j = 0:512, 512:4096, 4096:8192, matching _build's HXC)
    hx_tpj = hx.T.reshape(2, 128, N).transpose(1, 0, 2)  # [p, t, j]
    _chunks = [(0, 512), (512, 4096), (4096, 8192)]
    hx_dev = np.concatenate(
        [
            np.ascontiguousarray(hx_tpj[:, :, lo:hi]).reshape(128, 2 * (hi - lo))
            for lo, hi in _chunks
        ],
        axis=1,
    )

    hT16 = np.ascontiguousarray(h.T).astype(np.float16)
    WT16 = np.ascontiguousarray(W.T).astype(np.float16)
    bvf = b.reshape(1, 256).astype(np.float16).copy()

    hnewb = (h @ W.T + b).astype(np.float32)
    hn8 = hnewb.astype(E5).view(np.uint8)  # [N, 256]
    one5 = np.float32(1.0).astype(E5).view(np.uint8).item()
    hn_pair = np.empty((128, NJC // 2, 2, 257), np.uint8)
    hnr = hn8.reshape(NJC, 128, 256)  # [jc, p, f]
    hn_pair[:, :, 0, 0:256] = hnr[0::2].transpose(1, 0, 2)
    hn_pair[:, :, 1, 0:256] = hnr[1::2].transpose(1, 0, 2)
    hn_pair[:, :, :, 256] = one5
    hn_dev = np.ascontiguousarray(hn_pair.reshape(128, (NJC // 2) * 514))

    adjb = adj != 0
    keepval = np.where(np.asarray(CLS_A), 123, 1).astype(np.int8)  # [NJC]

    # diag term d (exact, host): em scale K = 1/2
    satt_ii = (h8dec[:, 0:255] ** 2).sum(axis=1, dtype=np.float32)
    diag_m = np.asarray(adjb.diagonal())
    d_all = np.where(
        diag_m, np.exp(satt_ii.astype(np.float64) / SLOPE) * 0.5, 0.0
    ).astype(np.float32)

    cbias8 = np.float32(CBIAS).astype(E4).view(np.uint8).item()

    in_maps = []
    row_lists = []
    for c in range(NCORES):
        id_rows = np.arange(c * kid, (c + 1) * kid)
        id_valid = id_rows < k
        id_rows = np.where(id_valid, id_rows, 0)
        att_rows = np.arange(k + c * rpad, k + (c + 1) * rpad)
        att_valid = att_rows < N
        att_rows_c = np.where(att_valid, att_rows, 0)
        rows = np.concatenate([id_rows, att_rows_c])
        row_lists.append((id_rows, id_valid, att_rows_c, att_valid))

        hxo = np.empty((256, rpad), np.uint8)  # [d, i]
        hxo[:, :] = hx[att_rows_c, :].T
        hxo[255, :] = cbias8
        hxo_dev = np.ascontiguousarray(
            hxo.reshape(2, 128, rpad).transpose(1, 0, 2)
        ).reshape(128, 2 * rpad)

        hTo = np.ascontiguousarray(hT16[:, rows])  # [D, own] f16

        # maskC [jc, p, i] -> [128 p, jc*i]
        msel = adjb[att_rows_c, :].T  # [N j, rpad i]
        mjc = msel.reshape(NJC, 128, rpad)
        mC = np.where(mjc, keepval[:, None, None], 0).astype(np.int8)
        # zero invalid (padded) i columns and the diagonal
        if not att_valid.all():
            mC[:, :, ~att_valid] = 0
        jj = att_rows_c
        jc_idx = jj // 128
        p_idx = jj % 128
        i_idx = np.arange(rpad)
        mC[jc_idx[att_valid], p_idx[att_valid], i_idx[att_valid]] = 0
        mC_dev = np.ascontiguousarray(mC.transpose(1, 0, 2)).reshape(
            128, NJC * rpad
        )

        dv = np.zeros((128, nis), np.float32)
        dvals = np.where(att_valid, d_all[att_rows_c], 0.0).astype(np.float32)
        dv[:, :] = dvals.reshape(nis, 128).T

        im = {
            "hx": hx_dev,
            "hxo": hxo_dev,
            "hTo": hTo,
            "WT": WT16,
            "bv": bvf,
            "hn": hn_dev,
            "mT": mC_dev,
            "dv": dv,
        }
        in_maps.append(im)

    def assemble(outs):
        out = np.empty((N, 256), dtype=np.float32)
        for c in range(NCORES):
            id_rows, id_valid, att_rows_c, att_valid = row_lists[c]
            o = outs[c]
            if id_valid.any():
                out[id_rows[id_valid]] = o[:kid][id_valid]
            if att_valid.any():
                out[att_rows_c[att_valid]] = o[kid:][att_valid]
        return out

    return nc, in_maps, assemble


def kernel(h, adj, W, b):
    nc, in_maps, assemble = _prepare(h, adj, W, b)

    from concourse.bass_utils import run_bass_kernel_spmd

    res = run_bass_kernel_spmd(nc, in_maps, core_ids=list(range(NCORES)))
    return assemble([res.results[c]["out"] for c in range(NCORES)])


# revision 17
# speedup vs baseline: 1.0686x; 1.0686x over previous
"""GAT layer kernel for Trainium2 (8 NeuronCores, SPMD, no collectives).

Math (reference):
    att = h @ h.T / sqrt(256)
    A = softmax(where(adj>0, att, -9e15), axis=1)
    A = (A + I) * 0.5; rows < k (k = nnz(adj[:,0])) overwritten with I
    out = relu(A @ (h @ W.T + b))

v2 algorithm (fp8 DoubleRow matmuls + exp-as-fp8-bit-encode):
  - rows [0,k): out = relu(h@W.T + b)  (identity rows)
  - rows [k,N): out = relu(num*(0.5/S_c) + g2*(1 + d/S_c)), where
        num[i,:] = sum_j em[j,i] * hnew8[j,:],  S = sum_j em[j,i],
        S_c = S + d_i,  g2 = 0.5*(h@W.T) + 0.5*b,
        d_i = host-computed exact diag term (m_ii * e^att_ii / 2).
  - em[j,i] ~= e^att[j,i]/2 stored as fp8e5 BIT PATTERNS: the PE computes
    satt = 5.7708*att + 56 directly (h8 pre-scaled by sqrt(5.7708/16);
    feature dim 255 is sacrificed for a constant bias row), and the u8
    bits b = satt decode in e5m2 as 2^((b-60)/4) = e^att * 2^-1.  The
    diagonal is excluded via the mask and re-blended exactly from host d.
  - att matmul: fp8e4 DoubleRow (K=256 in one pass, 0.5 cyc/col);
    second matmul: fp8e5 DoubleRow over j-chunk pairs.
  - mask fused into the encode, split across engines per j-chunk:
      class A (DVE):      em_u8 = min(att_psum, maskC{0,123})
      class B (ACT+Pool): em_u8 = Copy(att_psum)->u8; em *= m01{0,1}
  - hnew8 = fp8e5(h@W.T + b) precomputed host-side, streamed from DRAM.

Sharding: identity rows and attention rows each split evenly across the 8
cores; every core runs the same NEFF on different input slices.
"""

import math
import os
import sys

for _p in ("/opt/trn_rl_repo", "/root/.axon_site/_ro/trn_rl_repo"):
    if os.path.isdir(_p) and _p not in sys.path:
        sys.path.append(_p)

import numpy as np
import orjson

import concourse.bass as bass
import concourse.tile as tile
from concourse import mybir

F32 = mybir.dt.float32
F16 = mybir.dt.float16
BF16 = mybir.dt.bfloat16
F8E4 = mybir.dt.float8e4
F8E5 = mybir.dt.float8e5
I8 = mybir.dt.int8
U8 = mybir.dt.uint8
DR = mybir.MatmulPerfMode.DoubleRow

N = 8192
D = 256
NCORES = 8
NJC = N // 128  # 64 j-chunks
SLOPE = 8.0 / np.log(2.0) / 2.0  # 5.7708: e5m2 bit-steps per unit att
ALPHA = float(np.sqrt(SLOPE / 16.0))  # h8 pre-scale
CBIAS = 56.0  # bias row constant: bits = 5.7708*att + 56
# per-jc engine class: True -> class A (DVE fused min), False -> B (ACT+Pool)
CLS_A = [(jc % 2 == 0) or ((jc // 2) % 4 == 3) for jc in range(NJC)]


def _spill_waits(nc, max_sync=2):
    """Walrus rejects instructions with more sync commands than the lowered
    ISA struct can hold (2 for compute/DMA, 1 for NoOp/Drain/Ldweights).
    Move excess waits onto injected NoOps preceding the instruction."""
    bir = orjson.loads(nc.to_json_bytes())
    for fn in bir["functions"]:
        for bb in fn["blocks"]:
            insts = bb.get("instructions") or []
            out = []
            for inst in insts:
                si = inst.get("sync_info")
                if si:
                    waits = si.get("on_wait") or []
                    upds = si.get("on_update") or []
                    op = inst["opcode"]
                    lim = 1 if op in ("NoOp", "Drain", "Ldweights") else max_sync
                    cap = max(0, lim - len(upds))
                    if len(waits) > cap:
                        extra = waits[cap:]
                        si["on_wait"] = waits[:cap]
                        for ci, w in enumerate(extra):
                            out.append(
                                {
                                    "engine": inst["engine"],
                                    "ins": [],
                                    "outs": [],
                                    "name": f"{inst['name']}-sw{ci}",
                                    "opcode": "NoOp",
                                    "sync_info": {"on_wait": [w], "on_update": []},
                                    "debug": inst.get("debug", 0),
                                }
                            )
                out.append(inst)
            bb["instructions"] = out
    blob = orjson.dumps(bir)
    nc.to_json_bytes = lambda: blob


def _build(nid, nis, depth=3):
    """SPMD program. nid/nis = number of 128-row identity / attention
    sub-tiles per core. own = (nid+nis)*128 rows per core."""
    nown = nid + nis
    own = nown * 128
    rpad = nis * 128

    nc = bass.Bass("TRN2", target_bir_lowering=False, debug=False, num_devices=1)

    hx_d = nc.dram_tensor("hx", [128, 2 * N], U8, kind="ExternalInput").ap()
    hxo_d = nc.dram_tensor("hxo", [128, 2 * rpad], U8, kind="ExternalInput").ap()
    hTo_d = nc.dram_tensor("hTo", [D, own], F16, kind="ExternalInput").ap()
    WT_d = nc.dram_tensor("WT", [D, 256], F16, kind="ExternalInput").ap()
    bv_d = nc.dram_tensor("bv", [1, 256], F16, kind="ExternalInput").ap()
    hn_d = nc.dram_tensor("hn", [128, (NJC // 2) * 514], U8, kind="ExternalInput").ap()
    mT_d = nc.dram_tensor("mT", [128, NJC * rpad], I8, kind="ExternalInput").ap()
    dv_d = nc.dram_tensor("dv", [128, nis], F32, kind="ExternalInput").ap()
    out_d = nc.dram_tensor("out", [own, 256], BF16, kind="ExternalOutput").ap()

    with tile.TileContext(nc) as tc:
        with (
            tc.tile_pool(name="big", bufs=1) as big,
            tc.tile_pool(name="work", bufs=2) as work,
            tc.tile_pool(name="fin", bufs=2) as fin,
            tc.tile_pool(name="app", bufs=1, space="PSUM") as app,
            tc.tile_pool(name="accp", bufs=1, space="PSUM") as accp,
        ):
            # --- persistent loads (ordered for pipelining; separate tiles
            # per chunk because Tile dependency tracking is tile-granular) ---
            # hx j-chunks: [512, 3584, 4096]; mask jc-chunks; hn pair-chunks
            HXC = [(0, 512), (512, 4096), (4096, 8192)]
            MC = [(0, 2), (2, 8), (8, 16), (16, 24), (24, 32), (32, 40), (40, 48), (48, 56), (56, 64)]
            HC = [(0, 4), (4, 12), (12, 22), (22, 32)]
            hx_ts = {}
            mT_ts = {}
            hn_ts = {}

            def load_hx(ci):
                lo, hi = HXC[ci]
                t = big.tile([128, 2 * (hi - lo)], U8, tag=f"hx{ci}")
                nc.sync.dma_start(t[:], hx_d[:, 2 * lo : 2 * hi])
                hx_ts[ci] = t

            def load_m(ci):
                lo, hi = MC[ci]
                t = big.tile([128, (hi - lo) * rpad], I8, tag=f"mT{ci}")
                nc.sync.dma_start(t[:], mT_d[:, lo * rpad : hi * rpad])
                mT_ts[ci] = t

            def load_hn(ci):
                lo, hi = HC[ci]
                t = big.tile([128, (hi - lo) * 514], U8, tag=f"hn{ci}")
                nc.sync.dma_start(t[:], hn_d[:, lo * 514 : hi * 514])
                hn_ts[ci] = t

            load_hx(0)
            hxo_t = big.tile([128, 2 * rpad], U8, tag="hxo")
            nc.sync.dma_start(hxo_t[:], hxo_d[:, :])
            load_m(0)
            load_hx(1)
            load_m(1)
            load_hn(0)
            load_m(2)
            load_hx(2)
            load_hn(1)
            load_m(3)
            hTo_t = []
            WT_t = []
            for dchunk in range(2):
                t = big.tile([128, own], F16, tag=f"hTo{dchunk}")
                nc.sync.dma_start(t[:], hTo_d[dchunk * 128 : (dchunk + 1) * 128, :])
                hTo_t.append(t)
                t = big.tile([128, 256], F16, tag=f"WT{dchunk}")
                nc.sync.dma_start(t[:], WT_d[dchunk * 128 : (dchunk + 1) * 128, :])
                WT_t.append(t)
            bv_t = big.tile([1, 256], F16, tag="bv")
            nc.sync.dma_start(bv_t[:], bv_d[:, :])
            dv_t = big.tile([128, nis], F32, tag="dv")
            nc.sync.dma_start(dv_t[:], dv_d[:, :])
            one_row = big.tile([1, 128], F16, tag="onerow")
            nc.vector.memset(one_row[:], 1.0)
            load_m(4)
            load_hn(2)
            load_m(5)
            load_m(6)
            load_hn(3)
            load_m(7)
            load_m(8)

            def hx_slice(jc):
                for ci, (lo, hi) in enumerate(HXC):
                    if lo <= jc * 128 < hi:
                        t = hx_ts[ci]
                        w = hi - lo
                        a3 = t[:].bitcast(F8E4).rearrange("p (t j) -> p t j", t=2)
                        return a3[:, :, jc * 128 - lo : (jc + 1) * 128 - lo]
                raise AssertionError

            def m_slice(jc):
                for ci, (lo, hi) in enumerate(MC):
                    if lo <= jc < hi:
                        return mT_ts[ci][:, (jc - lo) * rpad : (jc - lo + 1) * rpad]
                raise AssertionError

            def m_slice2(pair):
                jc = 2 * pair
                for ci, (lo, hi) in enumerate(MC):
                    if lo <= jc < hi:
                        return mT_ts[ci][:, (jc - lo) * rpad : (jc - lo + 2) * rpad]
                raise AssertionError

            def hn_slice(pair):
                for ci, (lo, hi) in enumerate(HC):
                    if lo <= pair < hi:
                        return hn_ts[ci][:, (pair - lo) * 514 : (pair - lo + 1) * 514]
                raise AssertionError

            ones_t = big.tile([128, 1], F32, tag="ones1")
            nc.vector.memset(ones_t[:], 1.0)
            zer_t = big.tile([128, 256], F32, tag="zer256")
            nc.vector.memset(zer_t[:], 0.0)
            ebias_t = big.tile([128, 1], F32, tag="ebias")
            nc.vector.memset(ebias_t[:], -10.396842)

            hxo3 = hxo_t[:].bitcast(F8E4).rearrange("p (t i) -> p t i", t=2)

            # --- own phase emitter (interleaved into the main loop) ---
            g_t = [None] * nis

            def emit_own(t_i):
                psw = app.tile([128, rpad], F32, tag="att_ps0")
                ps = psw[:, 0:256]
                for dchunk in range(2):
                    nc.tensor.matmul(
                        ps,
                        hTo_t[dchunk][:, t_i * 128 : (t_i + 1) * 128],
                        WT_t[dchunk][:],
                        start=(dchunk == 0),
                        stop=False,
                    )
                nc.tensor.matmul(ps, one_row[:], bv_t[:], start=False, stop=True)
                if t_i < nid:
                    o_t = fin.tile([128, 256], BF16, tag="ido")
                    nc.scalar.activation(
                        o_t[:], ps, mybir.ActivationFunctionType.Relu
                    )
                    nc.sync.dma_start(out_d[t_i * 128 : (t_i + 1) * 128, :], o_t[:])
                else:
                    g = big.tile([128, 256], F32, tag=f"g{t_i - nid}")
                    nc.scalar.activation(
                        g[:], ps, mybir.ActivationFunctionType.Copy, scale=0.5
                    )
                    g_t[t_i - nid] = g

            # --- attention main loop ---
            acc = []
            for s in range(nis):
                acc_t = accp.tile([128, 257], F32, tag=f"acc{s}")
                acc.append(acc_t)
            def emit_sec_one(pair, em_pair, sidx):
                em3 = em_pair[:].bitcast(F8E5).rearrange("p (t i) -> p t i", t=2)
                hn3 = hn_slice(pair).bitcast(F8E5).rearrange("p (t f) -> p t f", t=2)
                nc.tensor.matmul(
                    acc[sidx][:],
                    em3[:, :, sidx * 128 : (sidx + 1) * 128],
                    hn3,
                    start=(pair == 0),
                    stop=(pair == NJC // 2 - 1),
                    perf_mode=DR,
                )

            em_t = None
            own_next = 0
            secq = []
            for jc in range(NJC):
                if jc % 6 == 2 and jc >= 8 and own_next < nown:
                    emit_own(own_next)
                    own_next += 1
                half = jc % 2
                if half == 0:
                    em_t = work.tile([128, 2 * rpad], U8, tag=f"em{(jc // 2) % 6}")
                aps = app.tile([128, rpad], F32, tag=f"att_ps{jc % 4}")
                nc.tensor.matmul(
                    aps[:],
                    hx_slice(jc),
                    hxo3,
                    start=True,
                    stop=True,
                    perf_mode=DR,
                )
                # interleave pending sec matmuls between att matmuls so their
                # short (107ns) MMs follow a long one and the LDW hides
                npop = min(2, len(secq)) if jc >= 4 else 0
                for _ in range(npop):
                    emit_sec_one(*secq.pop(0))
                em_half = em_t[:, half * rpad : (half + 1) * rpad]
                m_sl = m_slice(jc)
                if CLS_A[jc]:
                    nc.vector.tensor_tensor(
                        em_half, aps[:], m_sl, op=mybir.AluOpType.min
                    )
                else:
                    eb = work.tile([128, rpad], BF16, tag=f"eb{jc % 4}")
                    nc.scalar.activation(
                        eb[:], aps[:], mybir.ActivationFunctionType.Exp,
                        scale=0.17328679, bias=ebias_t[:],
                    )
                    nc.gpsimd.tensor_tensor(
                        em_half.bitcast(F8E5), eb[:], m_sl, op=mybir.AluOpType.mult
                    )
                if half == 1:
                    for sidx in range(nis):
                        secq.append((jc // 2, em_t, sidx))
            while own_next < nown:
                emit_own(own_next)
                own_next += 1
            for item in secq:
                emit_sec_one(*item)

            # --- finalize per s-tile ---
            for s in range(nis):
                a = acc[s]
                sc = fin.tile([128, 1], F32, tag="sc")
                nc.vector.tensor_tensor(
                    sc[:], a[:, 256:257], dv_t[:, s : s + 1], op=mybir.AluOpType.add
                )
                r = fin.tile([128, 1], F32, tag="r")
                nc.vector.reciprocal(r[:], sc[:])
                r0 = fin.tile([128, 1], F32, tag="r0")
                nc.vector.tensor_scalar_mul(r0[:], r[:], 0.5)
                rd2 = fin.tile([128, 1], F32, tag="rd2")
                nc.vector.scalar_tensor_tensor(
                    rd2[:], dv_t[:, s : s + 1], r[:], ones_t[:],
                    op0=mybir.AluOpType.mult, op1=mybir.AluOpType.add,
                )
                t1 = fin.tile([128, 256], F32, tag="t1")
                nc.scalar.activation(
                    t1[:], g_t[s][:], mybir.ActivationFunctionType.Copy,
                    scale=rd2[:],
                )
                t2 = fin.tile([128, 256], F32, tag="t2")
                nc.vector.scalar_tensor_tensor(
                    t2[:], a[:, 0:256], r0[:], t1[:],
                    op0=mybir.AluOpType.mult, op1=mybir.AluOpType.add,
                )
                o_t = fin.tile([128, 256], BF16, tag="ao")
                nc.vector.tensor_scalar_max(o_t[:], t2[:], 0.0)
                nc.sync.dma_start(
                    out_d[(nid + s) * 128 : (nid + s + 1) * 128, :], o_t[:]
                )

    _spill_waits(nc)
    return nc


_CACHE = {}


def _prepare(h, adj, W, b):
    """Host-side sharding + fp8 encode prep. Returns (nc, in_maps, assemble)."""
    import ml_dtypes

    E4 = ml_dtypes.float8_e4m3fn
    E5 = ml_dtypes.float8_e5m2

    h = np.asarray(h, dtype=np.float32)
    adj = np.asarray(adj)
    W = np.asarray(W, dtype=np.float32)
    b = np.asarray(b, dtype=np.float32)

    k = int(np.count_nonzero(adj[:, 0]))
    nid = (k + NCORES * 128 - 1) // (NCORES * 128)
    nis = (N - k + NCORES * 128 - 1) // (NCORES * 128)
    key = (nid, nis)
    if key not in _CACHE:
        _CACHE[key] = _build(nid, nis)
    nc = _CACHE[key]

    kid = nid * 128
    rpad = nis * 128

    # fp8 h encode, bias row at d=255
    h8q = (ALPHA * h).astype(E4)  # [N, 256]
    h8dec = h8q.astype(np.float32)
    hx = np.empty((N, 256), np.uint8)
    hx[:, :] = h8q.view(np.uint8)
    hx[:, 255] = np.float32(1.0).astype(E4).view(np.uint8).item()
    # device layout: concat of per-chunk [128 p, 2 t, w j] blocks (chunks
    # at j = 0:512, 512:4096, 4096:8192, matching _build's HXC)
    hx_tpj = hx.T.reshape(2, 128, N).transpose(1, 0, 2)  # [p, t, j]
    _chunks = [(0, 512), (512, 4096), (4096, 8192)]
    hx_dev = np.concatenate(
        [
            np.ascontiguousarray(hx_tpj[:, :, lo:hi]).reshape(128, 2 * (hi - lo))
            for lo, hi in _chunks
        ],
        axis=1,
    )

    hT16 = np.ascontiguousarray(h.T).astype(np.float16)
    WT16 = np.ascontiguousarray(W.T).astype(np.float16)
    bvf = b.reshape(1, 256).astype(np.float16).copy()

    hnewb = (h @ W.T + b).astype(np.float32)
    hn8 = hnewb.astype(E5).view(np.uint8)  # [N, 256]
    one5 = np.float32(1.0).astype(E5).view(np.uint8).item()
    hn_pair = np.empty((128, NJC // 2, 2, 257), np.uint8)
    hnr = hn8.reshape(NJC, 128, 256)  # [jc, p, f]
    hn_pair[:, :, 0, 0:256] = hnr[0::2].transpose(1, 0, 2)
    hn_pair[:, :, 1, 0:256] = hnr[1::2].transpose(1, 0, 2)
    hn_pair[:, :, :, 256] = one5
    hn_dev = np.ascontiguousarray(hn_pair.reshape(128, (NJC // 2) * 514))

    adjb = adj != 0
    keepval = np.where(np.asarray(CLS_A), 123, 1).astype(np.int8)  # [NJC]

    # diag term d (exact, host): em scale K = 1/2
    satt_ii = (h8dec[:, 0:255] ** 2).sum(axis=1, dtype=np.float32)
    diag_m = np.asarray(adjb.diagonal())
    d_all = np.where(
        diag_m, np.exp(satt_ii.astype(np.float64) / SLOPE) * 0.5, 0.0
    ).astype(np.float32)

    cbias8 = np.float32(CBIAS).astype(E4).view(np.uint8).item()

    in_maps = []
    row_lists = []
    for c in range(NCORES):
        id_rows = np.arange(c * kid, (c + 1) * kid)
        id_valid = id_rows < k
        id_rows = np.where(id_valid, id_rows, 0)
        att_rows = np.arange(k + c * rpad, k + (c + 1) * rpad)
        att_valid = att_rows < N
        att_rows_c = np.where(att_valid, att_rows, 0)
        rows = np.concatenate([id_rows, att_rows_c])
        row_lists.append((id_rows, id_valid, att_rows_c, att_valid))

        hxo = np.empty((256, rpad), np.uint8)  # [d, i]
        hxo[:, :] = hx[att_rows_c, :].T
        hxo[255, :] = cbias8
        hxo_dev = np.ascontiguousarray(
            hxo.reshape(2, 128, rpad).transpose(1, 0, 2)
        ).reshape(128, 2 * rpad)

        hTo = np.ascontiguousarray(hT16[:, rows])  # [D, own] f16

        # maskC [jc, p, i] -> [128 p, jc*i]
        msel = adjb[att_rows_c, :].T  # [N j, rpad i]
        mjc = msel.reshape(NJC, 128, rpad)
        mC = np.where(mjc, keepval[:, None, None], 0).astype(np.int8)
        # zero invalid (padded) i columns and the diagonal
        if not att_valid.all():
            mC[:, :, ~att_valid] = 0
        jj = att_rows_c
        jc_idx = jj // 128
        p_idx = jj % 128
        i_idx = np.arange(rpad)
        mC[jc_idx[att_valid], p_idx[att_valid], i_idx[att_valid]] = 0
        mC_dev = np.ascontiguousarray(mC.transpose(1, 0, 2)).reshape(
            128, NJC * rpad
        )

        dv = np.zeros((128, nis), np.float32)
        dvals = np.where(att_valid, d_all[att_rows_c], 0.0).astype(np.float32)
        dv[:, :] = dvals.reshape(nis, 128).T

        im = {
            "hx": hx_dev,
            "hxo": hxo_dev,
            "hTo": hTo,
            "WT": WT16,
            "bv": bvf,
            "hn": hn_dev,
            "mT": mC_dev,
            "dv": dv,
        }
        in_maps.append(im)

    def assemble(outs):
        out = np.empty((N, 256), dtype=np.float32)
        for c in range(NCORES):
            id_rows, id_valid, att_rows_c, att_valid = row_lists[c]
            o = outs[c]
            if id_valid.any():
                out[id_rows[id_valid]] = o[:kid][id_valid]
            if att_valid.any():
                out[att_rows_c[att_valid]] = o[kid:][att_valid]
        return out

    return nc, in_maps, assemble


def kernel(h, adj, W, b):
    nc, in_maps, assemble = _prepare(h, adj, W, b)

    from concourse.bass_utils import run_bass_kernel_spmd

    res = run_bass_kernel_spmd(nc, in_maps, core_ids=list(range(NCORES)))
    return assemble([res.results[c]["out"] for c in range(NCORES)])
